# revision 1
# baseline (speedup 1.0000x reference)
"""Trainium2 Bass kernel for DocSenModel (embedding -> conv sentence reps ->
bidirectional gated GNN chain -> softmax head).

Self-contained: takes FULL inputs, shards internally across 8 NeuronCores,
returns the FULL [5] output.  Written in raw Bass (explicit semaphores,
standalone waits - this toolchain's walrus allows at most one attached sync
wait per TPB instruction, so Tile-generated code does not compile).

Math refactoring (validated against the jax reference in fp32):
  * conv_k + avg-pool + tanh is linear before tanh, so each conv collapses to
    tiny [50x50] matmuls applied to per-sentence embedding-sum projections
    with edge corrections (words 0, 1, W-2, W-1).  All additive biases fold
    into host-precomputed ACT bias vectors (exact - they enter linearly).
  * The sequential 64-step bidirectional GNN recurrence is solved by
    Newton/Picard-Gauss-Seidel waveform iteration: gates are evaluated
    batched at the previous trajectory, tanh is linearized there, and the
    per-element linear recurrence h_t = a_t*h_{t-1} + b_t is solved exactly
    by one DVE tensor_tensor_scan per sweep.  5 sweeps reach ~1e-6 output
    accuracy (8 sweeps reach the 4.5e-8 fp32 noise floor).
  * softmax exp via the sigmoid identity exp(l) = 1/sigmoid(-l) - 1 so the
    whole kernel uses one ACT table set (sigmoid_and_others).

Sharding: data-parallel front-end (8 sentences/core: one 512-row indirect
gather each), AllGather of the tiny [8,50] reps, then the scan+head run
replicated on every core; core 0's output is returned.
"""

import os
import sys
from contextlib import ExitStack

import numpy as np

if "/opt/trn_rl_repo" not in sys.path:
    sys.path.insert(0, "/opt/trn_rl_repo")

import concourse.bass as bass
import concourse.mybir as mybir
from concourse.bass import IndirectOffsetOnAxis
from concourse.bass_utils import run_bass_kernel_spmd

F32 = mybir.dt.float32
I32 = mybir.dt.int32
AF = mybir.ActivationFunctionType
ALU = mybir.AluOpType

H = 50
E = 300
S = 64
W = 64
V = 100000
O = 5
NCORES = 8
SPC = S // NCORES
NSWEEP = 5

_COMPILED = {}


class Ctr:
    """Semaphore counter: tracks the expected value as instructions inc it."""

    def __init__(self, sem):
        self.sem = sem
        self.v = 0

    def inc(self, inst, n=1):
        inst.then_inc(self.sem, n)
        self.v += n
        return self.v


def _sel_groups(spc, ntile):
    """Selector-matmul output groups: rows = [sums(spc) | w0 | w1 | w62 | w63]
    split into chains of <=64 output rows (PE M limit / I64 slice)."""
    total = 5 * spc
    gs = []
    off = 0
    while off < total:
        gs.append((off, min(64, total - off)))
        off += min(64, total - off)
    return gs


def _layout(spc):
    """Column layout of the packed [128, NC] constant tensor."""
    ntile = spc * W // 128
    o = {}
    o["wx"] = 0                       # [128, 300]
    o["ij"] = 300                     # [64, 128]  I64 | J64
    o["wsel"] = 428                   # [128, ntile * 5*spc] selector lhsT
    o["wpk"] = o["wsel"] + ntile * 5 * spc   # [100, 155] wmain(150) whead(5)
    o["whb"] = o["wpk"] + 155         # [100, 303] wh(300) bgate(3)
    o["wcv"] = o["whb"] + 303         # [50, 304] conv(300) bconv(3) -b_out
    o["idx"] = o["wcv"] + 304         # [128, ntile] int32 bits
    o["ones"] = o["idx"] + ntile      # col rows0-4 =1 ; cols +1..+5 row0 =1
    o["end"] = o["ones"] + 6
    return o, ntile


def _build_nc(spc: int, nsweep: int):
    nc = bass.Bass(num_devices=NCORES, detect_race_conditions=False)
    sharded = spc != S
    L, ntile = _layout(spc)
    nrow = spc * W

    emb_d = nc.dram_tensor("emb", [V, E], F32, kind="ExternalInput")
    cst_d = nc.dram_tensor("cst", [128, L["end"]], F32, kind="ExternalInput")
    out_d = nc.dram_tensor("out", [O], F32, kind="ExternalOutput")
    if sharded:
        ccin_d = nc.dram_tensor("ccin", [spc, H], F32, kind="Internal")
        ccout_d = nc.dram_tensor("ccout", [S, H], F32, kind="Internal",
                                 addr_space="Shared")

    with ExitStack() as ctx:
        e = ctx.enter_context

        # ---- SBUF ----
        cst = e(nc.sbuf_tensor("cst_sb", [128, L["end"]], F32))
        ge = e(nc.sbuf_tensor("ge_sb", [128, E * ntile], F32))
        esum = e(nc.sbuf_tensor("esum_sb", [5 * spc if spc <= 25 else 128, E],
                                F32))
        esT = e(nc.sbuf_tensor("esT_sb", [100, 3 * 5 * spc], F32))
        ua = e(nc.sbuf_tensor("ua_sb", [50, 5 * spc], F32))
        m = e(nc.sbuf_tensor("m_sb", [50, 6 * spc], F32))
        tall = e(nc.sbuf_tensor("tall_sb", [50, 3 * spc], F32))
        reps = e(nc.sbuf_tensor("reps_sb", [50, spc], F32))
        ccin_sb = e(nc.sbuf_tensor("ccin_sb", [spc, H], F32))
        reps_sm = e(nc.sbuf_tensor("reps_sm_sb", [S, H], F32))
        xs = e(nc.sbuf_tensor("xs_sb", [128, S], F32))
        hbuf = e(nc.sbuf_tensor("hbuf_sb", [100, S + 1], F32))
        zi = e(nc.sbuf_tensor("zi_sb", [100, S], F32))
        zf = e(nc.sbuf_tensor("zf_sb", [100, S], F32))
        zg = e(nc.sbuf_tensor("zg_sb", [100, S], F32))
        c1 = e(nc.sbuf_tensor("c1_sb", [100, S], F32))
        c2 = e(nc.sbuf_tensor("c2_sb", [100, S], F32))
        st = e(nc.sbuf_tensor("st_sb", [100, S], F32))
        tt = e(nc.sbuf_tensor("tt_sb", [100, S], F32))
        qq = e(nc.sbuf_tensor("qq_sb", [100, S], F32))
        d1 = e(nc.sbuf_tensor("d1_sb", [100, S], F32))
        acf = e(nc.sbuf_tensor("ac_sb", [100, S], F32))
        bcf = e(nc.sbuf_tensor("bc_sb", [100, S], F32))
        hsum = e(nc.sbuf_tensor("hsum_sb", [100, 1], F32))
        sg5 = e(nc.sbuf_tensor("sg5_sb", [O, 1], F32))
        ex = e(nc.sbuf_tensor("ex_sb", [O, 1], F32))
        rs1 = e(nc.sbuf_tensor("rs1_sb", [1, 1], F32))
        probs = e(nc.sbuf_tensor("probs_sb", [O, 1], F32))

        # ---- PSUM: 8 tensors = 8 banks (2KB each).  One accumulation
        # group per bank at a time; start=True lazily zeroes its whole bank,
        # so a bank is only reused after its previous data is consumed. ----
        pA0 = e(nc.psum_tensor("pA0_ps", [128, 512], F32))
        pB0 = e(nc.psum_tensor("pB0_ps", [128, 512], F32))
        pC0 = e(nc.psum_tensor("pC0_ps", [128, 512], F32))
        pA1 = e(nc.psum_tensor("pA1_ps", [128, 512], F32))
        pB1 = e(nc.psum_tensor("pB1_ps", [128, 512], F32))
        pC1 = e(nc.psum_tensor("pC1_ps", [128, 512], F32))
        feb = e(nc.psum_tensor("feb_ps", [128, 512], F32))
        x7 = e(nc.psum_tensor("x7_ps", [128, 512], F32))

        # ---- semaphores ----
        sc = Ctr(e(nc.semaphore("sem_c")))      # const DMA
        sgt = [Ctr(e(nc.semaphore(f"sem_g{t}"))) for t in range(ntile)]
        sv = Ctr(e(nc.semaphore("sem_v")))      # DVE
        sa = Ctr(e(nc.semaphore("sem_a")))      # ACT
        sp = Ctr(e(nc.semaphore("sem_p")))      # PE
        sio = Ctr(e(nc.semaphore("sem_io")))    # misc DMA
        scc = Ctr(e(nc.semaphore("sem_cc")))    # collective

        # const slices
        wx = cst[:, L["wx"] : L["wx"] + 300]
        ij = cst[0:64, L["ij"] : L["ij"] + 128]
        wsel = cst[:, L["wsel"] : L["wsel"] + ntile * 5 * spc]
        wpk = cst[0:100, L["wpk"] : L["wpk"] + 155]
        wmain = wpk[:, 0:150]
        whead = wpk[:, 150:155]
        whb = cst[0:100, L["whb"] : L["whb"] + 303]
        wh = whb[:, 0:300]
        bgate = whb[:, 300:303]
        wcv = cst[0:50, L["wcv"] : L["wcv"] + 304]
        bconv = wcv[:, 300:303]
        bhead = cst[0:O, L["wcv"] + 303 : L["wcv"] + 304]
        idx = cst[:, L["idx"] : L["idx"] + ntile].bitcast(I32)
        ones51 = cst[0:O, L["ones"] : L["ones"] + 1]
        ones15 = cst[0:1, L["ones"] + 1 : L["ones"] + 6]

        # ================= const load + gather =================
        # indices first (tiny) so the gathers start immediately; bulk after
        sc.inc(nc.sync.dma_start(cst[:, L["idx"] :], cst_d[:, L["idx"] :]), 16)
        sc.inc(nc.sync.dma_start(cst[:, 0 : L["idx"]], cst_d[:, 0 : L["idx"]]), 16)

        # preload the ACT function table (sigmoid_and_others) off the
        # critical path: dummy op on junk data right after the first DMA
        nc.scalar.wait_ge(sc.sem, 16)
        nc.scalar.activation(sg5[0:1, 0:1], cst[0:1, L["idx"] : L["idx"] + 1],
                             AF.Tanh)

        nc.gpsimd.wait_ge(sc.sem, 16)
        # per-tile gathers: 128 rows each, row-per-partition (standard layout)
        for t in range(ntile):
            sgt[t].inc(
                nc.gpsimd.indirect_dma_start(
                    out=ge[:, E * t : E * t + E],
                    out_offset=None,
                    in_=emb_d[:],
                    in_offset=IndirectOffsetOnAxis(ap=idx[:, t : t + 1], axis=0),
                ),
                16,
            )

        # ================= front-end =================
        # PE: selector matmuls: rows = [e_sum(spc) | w0 | w1 | w62 | w63]
        groups = _sel_groups(spc, ntile)
        nc.tensor.wait_ge(sc.sem, 32)
        nsel = 5 * spc
        for t in range(ntile):
            nc.tensor.wait_ge(sgt[t].sem, 16)
            for gi, (goff, gcnt) in enumerate(groups):
                i_ = nc.tensor.matmul(
                    feb[goff : goff + gcnt, 0:E],
                    lhsT=wsel[:, t * nsel + goff : t * nsel + goff + gcnt],
                    rhs=ge[:, E * t : E * t + E],
                    start=(t == 0), stop=(t == ntile - 1))
        v_sel = sp.inc(i_)

        # DVE: PSUM -> SBUF
        nc.vector.wait_ge(sp.sem, v_sel)
        v_es = sv.inc(nc.vector.tensor_copy(esum[0:nsel, :], feb[0:nsel, 0:E]))

        # PE: transpose E-chunks ([nsel,100] -> [100,nsel]) into 3 banks
        tbanks = [pA0, pB0, pC0]
        nc.tensor.wait_ge(sv.sem, v_es)
        for j in range(3):
            for goff, gcnt in groups:
                i_ = nc.tensor.matmul(
                    tbanks[j][0:100, goff : goff + gcnt],
                    lhsT=esum[goff : goff + gcnt, 100 * j : 100 * j + 100],
                    rhs=ij[0:gcnt, 0:gcnt],
                    start=True, stop=True)
        v_tr = sp.inc(i_)
        nc.vector.wait_ge(sp.sem, v_tr)
        for j in range(3):
            i_ = nc.vector.tensor_copy(esT[:, j * nsel : (j + 1) * nsel],
                                       tbanks[j][0:100, 0:nsel])
        v_esT = sv.inc(i_)

        # PE: projection: ua = W_word @ [e_sum | boundaries]  [50, nsel]
        nc.tensor.wait_ge(sv.sem, v_esT)
        for j in range(3):
            i_ = nc.tensor.matmul(feb[0:50, 0:nsel],
                                  lhsT=wmain[:, 50 * j : 50 * j + 50],
                                  rhs=esT[:, j * nsel : (j + 1) * nsel],
                                  start=(j == 0), stop=(j == 2))
        v_fe = sp.inc(i_)

        # DVE: copy + m vectors
        nc.vector.wait_ge(sp.sem, v_fe)
        sv.inc(nc.vector.tensor_copy(ua[:], feb[0:50, 0:nsel]))
        nc.vector.wait_ge(sv.sem, sv.v)   # DVE write-ack before same-engine read
        sall = ua[:, 0:spc]
        u0 = ua[:, spc : 2 * spc]
        u1 = ua[:, 2 * spc : 3 * spc]
        u62 = ua[:, 3 * spc : 4 * spc]
        u63 = ua[:, 4 * spc : 5 * spc]
        ms = [m[:, k * spc : (k + 1) * spc] for k in range(6)]
        nc.vector.tensor_copy(ms[0], sall)
        nc.vector.tensor_tensor(ms[1], sall, u63, op=ALU.subtract)
        i_ = nc.vector.tensor_tensor(ms[2], sall, u0, op=ALU.subtract)
        sv.inc(i_)
        nc.vector.wait_ge(sv.sem, sv.v)   # ack before ms[3..5] read ms[1],ms[2]
        nc.vector.tensor_tensor(ms[3], ms[1], u62, op=ALU.subtract)
        nc.vector.tensor_tensor(ms[4], ms[2], u63, op=ALU.subtract)
        v_m = sv.inc(nc.vector.tensor_tensor(ms[5], ms[2], u1, op=ALU.subtract))

        # PE: conv matmuls, one bank per conv-kernel group
        cbank = [pA1, pB1, pC1]
        nc.tensor.wait_ge(sv.sem, v_m)
        plan = [(0, 0, True, True), (1, 1, True, False), (2, 1, False, True),
                (3, 2, True, False), (4, 2, False, False), (5, 2, False, True)]
        for k, grp, st_, sp_ in plan:
            i_ = nc.tensor.matmul(cbank[grp][0:50, 0:spc],
                                  lhsT=wcv[:, 50 * k : 50 * k + 50],
                                  rhs=ms[k], start=st_, stop=sp_)
        v_c = sp.inc(i_)

        # ACT: tanh over conv groups (bias consts need the bulk DMA)
        nc.scalar.wait_ge(sc.sem, 32)
        nc.scalar.wait_ge(sp.sem, v_c)
        for grp in range(3):
            i_ = nc.scalar.activation(tall[:, grp * spc : (grp + 1) * spc],
                                      cbank[grp][0:50, 0:spc],
                                      AF.Tanh, bias=bconv[:, grp : grp + 1])
        v_tall = sa.inc(i_)

        # DVE: reps = t1+t2+t3
        nc.vector.wait_ge(sa.sem, v_tall)
        sv.inc(nc.vector.tensor_tensor(reps[:], tall[:, 0:spc],
                                       tall[:, spc : 2 * spc], op=ALU.add))
        nc.vector.wait_ge(sv.sem, sv.v)
        v_reps = sv.inc(nc.vector.tensor_tensor(
            reps[:], reps[:], tall[:, 2 * spc : 3 * spc], op=ALU.add))

        # ================= reps exchange =================
        if sharded:
            # PE-transpose reps [50, spc] -> [spc, 50] for a contiguous store
            nc.tensor.wait_ge(sv.sem, v_reps)
            v_t = sp.inc(nc.tensor.matmul(x7[0:spc, 0:50], lhsT=reps[:],
                                          rhs=ij[0:50, 0:50],
                                          start=True, stop=True))
            nc.vector.wait_ge(sp.sem, v_t)
            v_ci = sv.inc(nc.vector.tensor_copy(ccin_sb[:], x7[0:spc, 0:50]))
            nc.sync.wait_ge(sv.sem, v_ci)
            sio.inc(nc.sync.dma_start(ccin_d[:], ccin_sb[:]), 16)
            nc.gpsimd.wait_ge(sio.sem, 16)
            scc.inc(nc.gpsimd.collective_compute(
                "AllGather", ALU.bypass,
                replica_groups=[list(range(NCORES))],
                ins=[ccin_d[:]], outs=[ccout_d[:]]))
            nc.sync.wait_ge(scc.sem, 1)
            sio.inc(nc.sync.dma_start(reps_sm[:], ccout_d[:]), 16)
            nc.tensor.wait_ge(sio.sem, sio.v)
        else:
            # transpose reps [50,64] -> reps_sm [64,50] via identity matmul
            nc.tensor.wait_ge(sv.sem, v_reps)
            v_t = sp.inc(nc.tensor.matmul(x7[0:64, 0:50], lhsT=reps[:],
                                          rhs=ij[0:50, 0:50],
                                          start=True, stop=True))
            nc.vector.wait_ge(sp.sem, v_t)
            v_cp = sv.inc(nc.vector.tensor_copy(reps_sm[:], x7[0:64, 0:50]))
            nc.tensor.wait_ge(sv.sem, v_cp)

        # PE: X = reps^T (bank x7) and col-reversed reps^T (bank feb)
        nc.tensor.matmul(x7[0:50, 64:128], lhsT=reps_sm[:], rhs=ij[:, 0:64],
                         start=True, stop=True)
        v_xfb = sp.inc(nc.tensor.matmul(feb[0:50, 64:128], lhsT=reps_sm[:],
                                        rhs=ij[:, 64:128], start=True, stop=True))

        # DVE: build X_stack (zero pad rows; fwd 0-49, bwd 64-113)
        nc.vector.memset(xs[:], 0.0)
        nc.vector.memset(hbuf[:], 0.0)
        nc.vector.wait_ge(sp.sem, v_xfb)
        nc.vector.tensor_copy(xs[0:50, :], x7[0:50, 64:128])
        v_xs = sv.inc(nc.vector.tensor_copy(xs[64:114, :], feb[0:50, 64:128]))

        # ================= Newton-GS sweeps =================
        hp = hbuf[:, 0:S]
        gbanks = [[pA0, pB0, pC0], [pA1, pB1, pC1]]
        v_scan = v_xs
        v_z = [0, 0]   # sa value of the last gate-ACT that read parity i

        # prologue x-matmuls for sweep 0 (h_0 = 0, so these close the
        # accumulation groups directly - no h-matmuls in sweep 0)
        nc.tensor.wait_ge(sv.sem, v_xs)
        vg0 = [0, 0, 0]
        for a in (0, 2, 1):
            i_ = nc.tensor.matmul(gbanks[0][a][0:100, 0:64],
                                  lhsT=wx[:, 100 * a : 100 * a + 100], rhs=xs[:],
                                  start=True, stop=True)
            vg0[a] = sp.inc(i_)

        for k in range(nsweep):
            pre = gbanks[k % 2]
            nxt = gbanks[(k + 1) % 2]
            if k == 0:
                vg = vg0
                v_hmm = 0
            else:
                # PE: accumulate Wh @ h_prev (waits previous scan)
                nc.tensor.wait_ge(sv.sem, v_scan)
                vg = [0, 0, 0]
                for a in (0, 2, 1):
                    i_ = nc.tensor.matmul(pre[a][0:100, 0:64],
                                          lhsT=wh[:, 100 * a : 100 * a + 100],
                                          rhs=hp, start=False, stop=True)
                    vg[a] = sp.inc(i_)
                v_hmm = vg[1]
            # PE: hoisted x-matmuls for the next sweep (WAR: gates of sweep
            # k-1 must have consumed nxt first)
            if k + 1 < nsweep:
                nc.tensor.wait_ge(sa.sem, v_z[(k + 1) % 2])
                for a in range(3):
                    nc.tensor.matmul(nxt[a][0:100, 0:64],
                                     lhsT=wx[:, 100 * a : 100 * a + 100],
                                     rhs=xs[:], start=True,
                                     stop=(k + 1 == nsweep - 1 and False)
                                     or False)

            # ACT: gates zi, zg, zf (matches h-MM order)
            nc.scalar.wait_ge(sv.sem, v_scan)
            nc.scalar.wait_ge(sp.sem, vg[0])
            nc.scalar.activation(zi[:], pre[0][0:100, 0:64], AF.Sigmoid,
                                 bias=bgate[:, 0:1])
            nc.scalar.wait_ge(sp.sem, vg[2])
            v_zg = sa.inc(nc.scalar.activation(zg[:], pre[2][0:100, 0:64],
                                               AF.Tanh, bias=bgate[:, 2:3]))
            nc.scalar.wait_ge(sp.sem, vg[1])
            v_zf = sa.inc(nc.scalar.activation(zf[:], pre[1][0:100, 0:64],
                                               AF.Sigmoid, bias=bgate[:, 1:2]))
            v_z[k % 2] = v_zf

            # DVE: s~ = zi*zg + zf*hp
            nc.vector.wait_ge(sa.sem, v_zg)
            nc.vector.tensor_tensor(c1[:], zi[:], zg[:], op=ALU.mult)
            nc.vector.wait_ge(sa.sem, v_zf)
            sv.inc(nc.vector.tensor_tensor(c2[:], zf[:], hp, op=ALU.mult))
            nc.vector.wait_ge(sv.sem, sv.v)
            v_st = sv.inc(nc.vector.tensor_tensor(st[:], c1[:], c2[:],
                                                  op=ALU.add))

            # ACT: T
            nc.scalar.wait_ge(sv.sem, v_st)
            v_tt = sa.inc(nc.scalar.activation(tt[:], st[:], AF.Tanh))

            # DVE: coefficients (b = (T - c2) + T^2*c2, a = zf - T^2*zf)
            nc.vector.wait_ge(sa.sem, v_tt)
            nc.vector.tensor_tensor(qq[:], tt[:], tt[:], op=ALU.mult)
            sv.inc(nc.vector.tensor_tensor(st[:], tt[:], c2[:], op=ALU.subtract))
            nc.vector.wait_ge(sv.sem, sv.v)
            nc.vector.tensor_tensor(d1[:], qq[:], c2[:], op=ALU.mult)
            sv.inc(nc.vector.tensor_tensor(c1[:], qq[:], zf[:], op=ALU.mult))
            nc.vector.wait_ge(sv.sem, sv.v)
            nc.vector.tensor_tensor(bcf[:], st[:], d1[:], op=ALU.add)
            sv.inc(nc.vector.tensor_tensor(acf[:], zf[:], c1[:],
                                           op=ALU.subtract))
            nc.vector.wait_ge(sv.sem, sv.v)
            nc.vector.wait_ge(sp.sem, v_hmm)   # WAR: PE read of hp done
            v_scan = sv.inc(nc.vector.tensor_tensor_scan(
                hbuf[:, 1 : S + 1], acf[:], bcf[:], initial=0.0,
                op0=ALU.mult, op1=ALU.add))

        # ================= head =================
        v_hsum = sv.inc(nc.vector.reduce_sum(hsum[:], hbuf[:, 1 : S + 1],
                                             axis=mybir.AxisListType.X))
        nc.tensor.wait_ge(sv.sem, v_hsum)
        v_lg = sp.inc(nc.tensor.matmul(feb[0:O, 0:1], lhsT=whead[:], rhs=hsum[:],
                                       start=True, stop=True))
        nc.scalar.wait_ge(sp.sem, v_lg)
        v_sg = sa.inc(nc.scalar.activation(sg5[:], feb[0:O, 0:1], AF.Sigmoid,
                                           scale=-1.0, bias=bhead))
        nc.vector.wait_ge(sa.sem, v_sg)
        sv.inc(nc.vector.reciprocal(ex[:], sg5[:]))
        nc.vector.wait_ge(sv.sem, sv.v)
        v_ex = sv.inc(nc.vector.tensor_scalar(ex[:], ex[:], -1.0, None,
                                              op0=ALU.add))
        nc.tensor.wait_ge(sv.sem, v_ex)
        v_sm = sp.inc(nc.tensor.matmul(x7[0:1, 0:1], lhsT=ones51, rhs=ex[:],
                                       start=True, stop=True))
        nc.vector.wait_ge(sp.sem, v_sm)
        v_rs = sv.inc(nc.vector.reciprocal(rs1[:], x7[0:1, 0:1]))
        nc.tensor.wait_ge(sv.sem, v_rs)
        v_rb = sp.inc(nc.tensor.matmul(feb[0:O, 0:1], lhsT=ones15, rhs=rs1[:],
                                       start=True, stop=True))
        nc.vector.wait_ge(sp.sem, v_rb)
        nc.vector.wait_ge(sv.sem, v_ex)
        v_pr = sv.inc(nc.vector.tensor_tensor(probs[:], ex[:], feb[0:O, 0:1],
                                              op=ALU.mult))
        nc.sync.wait_ge(sv.sem, v_pr)
        sio.inc(nc.sync.dma_start(out_d[:], probs[:]), 16)
        nc.sync.wait_ge(sio.sem, sio.v)

    return nc


def _prep_consts(inputs, spc):
    f32 = np.float32
    L, ntile = _layout(spc)
    W_word = np.asarray(inputs["W_word"], f32)
    b_word = np.asarray(inputs["b_word"], f32)

    cst = np.zeros((128, L["end"]), f32)

    # selector lhsT: per gather-tile t, cols [sums(spc) | w0 | w1 | w62 | w63]
    rows_per = W // ntile
    for t in range(ntile):
        base = L["wsel"] + t * 5 * spc
        for s_ in range(spc):
            p0 = s_ * rows_per
            cst[p0 : p0 + rows_per, base + s_] = 1.0
        for g, w_ in enumerate((0, 1, W - 2, W - 1)):
            for s_ in range(spc):
                r = W * s_ + w_
                if r % ntile == t:
                    cst[r // ntile, base + spc + g * spc + s_] = 1.0

    # wx [128, 300]: per gate [128, 100]: fwd rows 0-49, bwd rows 64-113; /3
    # wh [100, 300] blockdiag + gate biases
    for a, g in enumerate("ifg"):
        Wf = np.asarray(inputs[f"Wf_{g}"], f32)
        Wb = np.asarray(inputs[f"Wb_{g}"], f32)
        cst[0:50, L["wx"] + 100 * a : L["wx"] + 100 * a + 50] = (Wf[:, :H] / 3.0).T
        cst[64:114, L["wx"] + 100 * a + 50 : L["wx"] + 100 * a + 100] = \
            (Wb[:, :H] / 3.0).T
        cst[0:50, L["whb"] + 100 * a : L["whb"] + 100 * a + 50] = Wf[:, H:].T
        cst[50:100, L["whb"] + 100 * a + 50 : L["whb"] + 100 * a + 100] = \
            Wb[:, H:].T
        cst[0:50, L["whb"] + 300 + a] = np.asarray(inputs[f"bf_{g}"], f32)
        cst[50:100, L["whb"] + 300 + a] = np.asarray(inputs[f"bb_{g}"], f32)

    # I64 | J64
    cst[0:64, L["ij"] : L["ij"] + 64] = np.eye(64, dtype=f32)
    cst[0:64, L["ij"] + 64 : L["ij"] + 128] = np.eye(64, dtype=f32)[::-1]

    # projection chunks (natural E order) + head
    for j in range(3):
        cst[0:100, L["wpk"] + 50 * j : L["wpk"] + 50 * j + 50] = \
            W_word[:, 100 * j : 100 * j + 100].T
    cst[0:100, L["wpk"] + 150 : L["wpk"] + 155] = \
        (np.asarray(inputs["W_out"], f32) / S).T

    # conv lhsT + effective biases + head bias
    w1 = np.asarray(inputs["conv_w1"], f32)
    w2 = np.asarray(inputs["conv_w2"], f32)
    w3 = np.asarray(inputs["conv_w3"], f32)
    convs = [w1[:, :, 0] / W, w2[:, :, 0] / (W - 1), w2[:, :, 1] / (W - 1),
             w3[:, :, 0] / (W - 2), w3[:, :, 1] / (W - 2), w3[:, :, 2] / (W - 2)]
    for k, c in enumerate(convs):
        cst[0:50, L["wcv"] + 50 * k : L["wcv"] + 50 * k + 50] = c.T
    cst[0:50, L["wcv"] + 300] = np.asarray(inputs["conv_b1"], f32) + w1.sum(2) @ b_word
    cst[0:50, L["wcv"] + 301] = np.asarray(inputs["conv_b2"], f32) + w2.sum(2) @ b_word
    cst[0:50, L["wcv"] + 302] = np.asarray(inputs["conv_b3"], f32) + w3.sum(2) @ b_word
    cst[0:O, L["wcv"] + 303] = -np.asarray(inputs["b_out"], f32)

    # ones
    cst[0:O, L["ones"]] = 1.0
    cst[0:1, L["ones"] + 1 : L["ones"] + 6] = 1.0
    return cst, L, ntile


def kernel(**inputs) -> np.ndarray:
    doc = np.asarray(inputs["doc"]).astype(np.int32)
    emb = np.asarray(inputs["emb"], np.float32)
    cst0, L, ntile = _prep_consts(inputs, SPC)

    key = (SPC, NSWEEP)
    if key not in _COMPILED:
        _COMPILED[key] = _build_nc(SPC, NSWEEP)
    nc = _COMPILED[key]

    in_maps = []
    for c in range(NCORES):
        sents = doc[c * SPC : (c + 1) * SPC] if SPC != S else doc
        cst = cst0.copy()
        cst[:, L["idx"] : L["idx"] + ntile] = \
            sents.reshape(128, ntile).view(np.float32)
        in_maps.append({"emb": emb, "cst": cst})

    res = run_bass_kernel_spmd(
        nc, in_maps, core_ids=list(range(NCORES)),
        trace=bool(int(os.environ.get("DOCSEN_TRACE", "0"))),
    )
    kernel.last_results = res
    return np.asarray(res.results[0]["out"], np.float32)

